# revision 62
# baseline (speedup 1.0000x reference)
"""Trainium2 Bass kernel for a dense transformer block (pre-LN, causal MHA + 4x MLP).

Sharding over 8 NeuronCores: attention is head-sharded 8 ways (each core does
H/8 heads for BOTH batches — identical causal structure on every core), then a
single 8-rank AllToAll re-shards activations to (batch, token-block) shards so
the out-projection and MLP run row-sharded with full weights (no all-reduce).

All on-chip activations are kept feature-major ("transposed": features on the
partition axis) so matmuls consume weights in their natural [in, out] layout
and no on-chip transposes are needed. The host supplies x pre-transposed and
re-transposes the output. LayerNorm statistics (partition-axis reductions) are
computed with ones-vector matmuls on the TensorEngine; softmax denominators
ride along as an appended ones-column in the attention-value matmul.

Precision: everything that crosses HBM (x, weights, attention y / AllToAll
payload, MLP hidden) is bf16 — matmul throughput is identical to f32r but DMA
bytes, SBUF footprint and DVE element costs halve. Attention q/k/scores and
the residual stream stay fp32(r) (same PE speed at free-dim >= 256).
"""
import numpy as np
from contextlib import ExitStack

import concourse.bass as bass
import concourse.mybir as mybir
import concourse.tile as tile
from concourse import bacc

F32 = mybir.dt.float32
F32R = mybir.dt.float32r
BF16 = mybir.dt.bfloat16
AF = mybir.ActivationFunctionType
ALU = mybir.AluOpType

# The act-table chooser resolves `ln` to the natural_log-only set and `exp`
# to exp_and_others, so a kernel alternating Ln/Exp reloads tables every
# chunk (~1.3us each). Hide ln/exp from the single-function sets (keeping
# dict order, so act_func_set_id still indexes act_info.json correctly) so
# the chooser lands on the combined natural_log_exp_and_others set.
import concourse.hw_specs as _hw_specs

_orig_gat = _hw_specs.get_activation_tables


def _gat(arch):
    t = dict(_orig_gat(arch))
    comb = t.get("natural_log_exp_and_others")
    if comb:
        for name in ("natural_log", "exp_and_others"):
            if name in t:
                t[name] = t[name] - {f for f in comb if str(f).endswith(("Ln", "Exp"))}
    return t


bacc.get_activation_tables = _gat


class Cfg:
    def __init__(self, D=1024, DFF=4096, H=16, T=2048, B=2, TP=4, HD=64,
                 mm_fast=True, gelu_native=True):
        self.gelu_native = gelu_native
        self.D, self.DFF, self.H, self.T, self.B, self.TP, self.HD = D, DFF, H, T, B, TP, HD
        self.NC = B * TP              # cores
        self.mm_fast = mm_fast
        self.KD = D // 128            # feature chunks
        self.FD = DFF // 128          # hidden chunks
        self.TCH = min(512, T)        # attention q-block width
        self.LCH = min(512, T)        # LN/QKV streaming token chunk
        self.NTL = T // self.LCH      # streaming chunks per batch
        self.NT = T // self.TCH       # token chunks per batch
        self.QB = self.NT             # attention q blocks per batch
        self.NKC = T // 128           # key chunks per batch
        self.NDIAG = self.TCH // 128  # diagonal masks
        self.NHC = H // self.NC      # heads per core (8-way head shard)
        self.HC = self.NHC * HD       # head feature columns per core
        self.HPT = min(2, self.NHC)   # heads per SBUF tile
        self.PT = self.HPT * HD       # partitions per head tile (64 or 128)
        self.HT = self.NHC // self.HPT  # head tiles per batch
        self.TQ = T // TP             # owned tokens per core
        self.OCB = min(512, D)        # output-feature column block
        self.HCB = min(512, DFF)      # hidden column block
        assert H % self.NC == 0 and T % TP == 0 and D % 128 == 0
        assert HD == 64 and self.TCH % 128 == 0 and DFF % 128 == 0


CFG = Cfg()


def emit(ctx: ExitStack, tc: tile.TileContext, io: dict, cfg: Cfg):
    nc = tc.nc
    c = cfg
    rearr = lambda ap: ap.rearrange("(o p) t -> p o t", p=128)

    def mm(ps, lhsT, rhs, start, stop):
        if c.mm_fast and lhsT.dtype == F32:
            lhsT = lhsT.bitcast(F32R)
        if c.mm_fast and rhs.dtype == F32:
            rhs = rhs.bitcast(F32R)
        nc.tensor.matmul(ps, lhsT, rhs, start=start, stop=stop)

    # writes into f32 matmul-feeding tiles must be rounded to f32r; bf16
    # tiles need nothing
    def rnd(ap):
        if c.mm_fast and ap.dtype == F32:
            return ap.bitcast(F32R)
        return ap

    # ---------------- constant / persistent pools ----------------
    const = ctx.enter_context(tc.tile_pool(name="const", bufs=1))
    small = ctx.enter_context(tc.tile_pool(name="small", bufs=1))
    bcast = ctx.enter_context(tc.tile_pool(name="bcast", bufs=1))
    pmm = ctx.enter_context(tc.tile_pool(name="pmm", bufs=3, space="PSUM"))
    dram = ctx.enter_context(tc.tile_pool(name="dram", bufs=1, space="DRAM"))

    ident = const.tile([128, 128], F32, tag="ident")
    from concourse.masks import make_identity
    make_identity(nc, ident[:])
    ones_f = const.tile([128, 1], F32, tag="ones_f")
    nc.gpsimd.memset(ones_f[:], 1.0)
    ones = const.tile([128, 1], F32, tag="ones")
    nc.vector.tensor_copy(rnd(ones[:]), ones_f[:])
    ones_b = const.tile([128, 1], BF16, tag="ones_b")
    nc.vector.tensor_copy(ones_b[:], ones_f[:])
    ones_r = const.tile([1, 128], F32, tag="ones_r")
    nc.vector.tensor_copy(rnd(ones_r[:]),
                          ones_f[0:1, 0:1].to_broadcast((1, 128)))
    eps_t = const.tile([1, 1], F32, tag="eps")
    nc.gpsimd.memset(eps_t[:], 1e-5)

    # causal masks for the diagonal 128-key blocks: mask_d[p, j] = (j >= 128d+p).
    # Applied post-exp with a DVE multiply — keeps the Pool engine (which the
    # 28us collectives head-block) off the attention critical path.
    cmask = []
    for d in range(c.NDIAG):
        mk = const.tile([128, c.TCH], BF16, tag=f"cmask{d}")
        nc.vector.tensor_copy(mk[:], ones_f[:, 0:1].to_broadcast((128, c.TCH)))
        nc.gpsimd.affine_select(
            out=mk[:], in_=mk[:], compare_op=ALU.is_ge, fill=0.0,
            base=-128 * d, pattern=[[1, c.TCH]], channel_multiplier=-1)
        cmask.append(mk)

    def pbcast(dst, row_ap, ncols, pool=None, tag="mm", nrows=128):
        """Broadcast [1, ncols] -> [nrows, ncols] via a K=1 PE matmul and a
        DVE copy (replaces gpsimd.partition_broadcast; Pool-free)."""
        ps = (pool or pmm).tile([128, c.TCH], F32, tag=tag,
                                name="psb")[:nrows, :ncols]
        mm(ps, ones_r[:, :nrows], row_ap, True, True)
        nc.vector.tensor_copy(dst, ps)

    nqp = min(128, c.HC)
    # pv = [bq | bk | bv | wsq | wsk | wsv] in partition 0 (bf16)
    pv = const.tile([1, 6 * c.HC], BF16, tag="pv", name="pv")
    nc.gpsimd.dma_start(pv[:], io["pv"][:])
    bqr, bkr, bvr, wsq, wsk, wsv = (
        pv[:, i * c.HC:(i + 1) * c.HC] for i in range(6))
    pm = const.tile([128, 2 * c.KD + c.FD], F32, tag="pm", name="pm")
    nc.gpsimd.dma_start(rnd(pm[:]), rnd(io["pm"][:]))
    bo = pm[:, 0:c.KD]
    bf1 = pm[:, c.KD:c.KD + c.FD]
    bf2 = pm[:, c.KD + c.FD:]

    # out-proj and fc1 weights are fully prefetched during phase 1 (SWDGE
    # queue) so phase 4 only streams wf2; DMAs emitted after the qkv weight
    # loads so they don't head-of-line block them
    wpre = ctx.enter_context(tc.tile_pool(name="wpre", bufs=1))
    wo_sb = wpre.tile([128, c.KD, c.D], BF16, tag="wo_sb")
    wf1_sb = wpre.tile([128, c.KD, c.DFF], BF16, tag="wf1_sb")

    inv_d = 1.0 / c.D

    def ln_stats_and_apply(xc, ncols, out, xsq_pool, tag):
        """xc: [128, KD, ncols] f32 raw input; out: normalized bf16."""
        ps1 = pmm.tile([128, c.TCH], F32, tag="mm", name="ps1")[:1, :ncols]
        for o in range(c.KD):
            mm(ps1, ones[:], xc[:, o, :], o == 0, o == c.KD - 1)
        ps2 = pmm.tile([128, c.TCH], F32, tag="mm", name="ps2")[:1, :ncols]
        for o in range(c.KD):
            xsq = xsq_pool.tile([128, ncols], F32, tag=f"xsq{tag}",
                                name="xsq")
            nc.vector.tensor_tensor(rnd(xsq), xc[:, o, :], xc[:, o, :], ALU.mult)
            mm(ps2, ones[:], xsq, o == 0, o == c.KD - 1)
        mu = small.tile([1, ncols], F32, tag="mu", name="mu")
        nc.vector.tensor_scalar_mul(mu, ps1, inv_d)
        ex2 = small.tile([1, ncols], F32, tag="ex2", name="ex2")
        nc.vector.tensor_scalar_mul(ex2, ps2, inv_d)
        var = small.tile([1, ncols], F32, tag="var", name="var")
        nc.vector.tensor_tensor(var, mu, mu, ALU.mult)
        nc.vector.tensor_tensor(var, ex2, var, ALU.subtract)
        lnv = small.tile([1, ncols], F32, tag="ex2", name="lnv")
        nc.scalar.activation(lnv, var, AF.Ln, bias=eps_t[:])
        A_ = small.tile([1, ncols], F32, tag="A", name="A_")
        nc.scalar.activation(rnd(A_), lnv, AF.Exp, scale=-0.5)
        B_ = small.tile([1, ncols], F32, tag="B", name="B_")
        nc.vector.scalar_tensor_tensor(rnd(B_), mu, -1.0, A_, ALU.mult, ALU.mult)
        Ab = bcast.tile([128, ncols], F32, tag="Ab", name="Ab")
        pbcast(Ab, A_, ncols)
        Bb = bcast.tile([128, ncols], F32, tag="Bb", name="Bb")
        pbcast(Bb, B_, ncols)
        for o in range(c.KD):
            nc.vector.tensor_tensor(out[:, o, :], xc[:, o, :], Ab, ALU.mult)
            nc.vector.tensor_tensor(out[:, o, :], out[:, o, :], Bb, ALU.add)

    def ln_stats(xc, ncols, xsq_pool, tag):
        """Column stats of bf16 [128, KD, ncols] tile -> (negmu, std, A_):
        negmu/std bf16 [1, ncols] (rank-1 matmul rhs), A_ = 1/std row
        (broadcast by the CALLER after it emits its independent matmuls —
        a pbcast emitted here would park the PE queue on the stats
        DVE/ACT chain)."""
        ps1 = pmm.tile([128, c.TCH], F32, tag="mm", name="ps1")[:1, :ncols]
        for o in range(c.KD):
            mm(ps1, ones_b[:], xc[:, o, :], o == 0, o == c.KD - 1)
        ps2 = pmm.tile([128, c.TCH], F32, tag="mm", name="ps2")[:1, :ncols]
        for o in range(c.KD):
            xsq = xsq_pool.tile([128, ncols], BF16, tag=f"xsq{tag}",
                                name="xsq")
            nc.vector.tensor_tensor(xsq, xc[:, o, :], xc[:, o, :], ALU.mult)
            mm(ps2, ones_b[:], xsq, o == 0, o == c.KD - 1)
        mu = small.tile([1, ncols], F32, tag="mu", name="mu")
        nc.vector.tensor_scalar_mul(mu, ps1, inv_d)
        ex2 = small.tile([1, ncols], F32, tag="ex2", name="ex2")
        nc.vector.tensor_scalar_mul(ex2, ps2, inv_d)
        var = small.tile([1, ncols], F32, tag="var", name="var")
        nc.vector.tensor_tensor(var, mu, mu, ALU.mult)
        nc.vector.tensor_tensor(var, ex2, var, ALU.subtract)
        # rsqrt/sqrt via Ln+Exp: with the table reorder above Ln and Exp
        # share one table with the attention exps -> no set reloads
        lnv = small.tile([1, ncols], F32, tag="ex2", name="lnv")
        nc.scalar.activation(lnv, var, AF.Ln, bias=eps_t[:])
        A_ = small.tile([1, ncols], F32, tag="A", name="A_")
        nc.scalar.activation(rnd(A_), lnv, AF.Exp, scale=-0.5)
        std_ = small.tile([1, ncols], BF16, tag="std", name="std_")
        nc.scalar.activation(std_, lnv, AF.Exp, scale=0.5)
        negmu = small.tile([1, ncols], BF16, tag="B", name="negmu")
        nc.vector.tensor_scalar_mul(negmu, mu, -1.0)
        return negmu, std_, A_

    # a2a dram bounce buffers, one exchange per batch: shard j = my
    # head-cols for batch-b tokens [j*TQH, (j+1)*TQH). After the AllToAll
    # block j = global head-cols [j*HC, ...) for MY TQH tokens of batch b.
    # Batch 0's exchange fires mid-kernel (after batch-0 attention) and
    # hides behind batch-1 compute; batch 1's hides behind the batch-0
    # half of out-proj/LN2/fc1.
    TQH = c.T // c.NC
    a2a_in = [dram.tile([c.NC * c.HC, TQH], BF16, tag=f"a2a_in{i}",
                        name=f"a2a_in{i}") for i in range(c.B)]
    a2a_out = [dram.tile([c.NC * c.HC, TQH], BF16, tag=f"a2a_out{i}",
                         name=f"a2a_out{i}") for i in range(c.B)]

    # phase-4 input staging lives at ctx scope so its DMAs can be emitted
    # mid-phase-1 (right after each batch's collective) instead of queueing
    # at the phase boundary
    oprjp = ctx.enter_context(tc.tile_pool(name="oprjp", bufs=1))
    yfull = [oprjp.tile([128, c.KD, TQH], BF16, tag=f"yfull{i}",
                        name=f"yfull{i}") for i in range(c.B)]
    xq = oprjp.tile([128, c.KD, 2 * TQH], BF16, tag="xq")

    # ================= phase 1+2: LN1, QKV, attention (per batch) ==========
    with ExitStack() as ph12:
        wqkv = ph12.enter_context(tc.tile_pool(name="wqkv", bufs=1))
        xcp = ph12.enter_context(tc.tile_pool(name="xcp", bufs=2))
        xsqp = ph12.enter_context(tc.tile_pool(name="xsqp", bufs=2))
        kvqy = ph12.enter_context(tc.tile_pool(name="kvqy", bufs=1))
        sp = ph12.enter_context(tc.tile_pool(name="sp", bufs=4))
        vtp = ph12.enter_context(tc.tile_pool(name="vtp", bufs=2))
        ps_s = ph12.enter_context(tc.tile_pool(name="ps_s", bufs=2, space="PSUM"))
        ps_y = ph12.enter_context(tc.tile_pool(name="ps_y", bufs=2, space="PSUM"))
        prb = ph12.enter_context(tc.tile_pool(name="prb", bufs=1, space="PSUM"))

        wq = wqkv.tile([128, c.KD, c.HC], BF16, tag="wq")
        nc.gpsimd.dma_start(wq[:], rearr(io["wq"]))
        wk = wqkv.tile([128, c.KD, c.HC], BF16, tag="wk")
        nc.gpsimd.dma_start(wk[:], rearr(io["wk"]))
        wv = wqkv.tile([128, c.KD, c.HC], BF16, tag="wv")
        nc.gpsimd.dma_start(wv[:], rearr(io["wv"]))

        # phase-4 weight prefetch, emitted piecewise at chunk boundaries so
        # the (serialized) DMA path never head-of-line blocks an xc load
        def _pre_wo(j2):
            return lambda: nc.gpsimd.dma_start(
                wo_sb[:, :, j2 * 512:(j2 + 1) * 512],
                rearr(io["wo"])[:, :, j2 * 512:(j2 + 1) * 512])

        def _pre_wf1(hcb):
            return lambda: nc.gpsimd.dma_start(
                wf1_sb[:, :, hcb * c.HCB:(hcb + 1) * c.HCB],
                rearr(io["wf1"])[:, :, hcb * c.HCB:(hcb + 1) * c.HCB])

        prefetch = [_pre_wo(0), _pre_wo(1)] + \
            [_pre_wf1(h) for h in range(c.DFF // c.HCB)]
        npre = len(prefetch)

        kT = [[kvqy.tile([c.PT, c.T], F32, tag=f"kT{b}_{i}", name=f"kT{b}_{i}")
               for i in range(c.HT)] for b in range(c.B)]
        qT = [[kvqy.tile([c.PT, c.T], F32, tag=f"qT{b}_{i}", name=f"qT{b}_{i}")
               for i in range(c.HT)] for b in range(c.B)]
        yT = [[kvqy.tile([c.PT, c.T], BF16, tag=f"yT{b}_{i}", name=f"yT{b}_{i}")
               for i in range(c.HT)] for b in range(c.B)]
        v_sb = [[kvqy.tile([128, c.NHC * 65], BF16, tag=f"v{b}_{a}",
                           name=f"v{b}_{a}") for a in range(c.NKC)]
                for b in range(c.B)]
        for b in range(c.B):
            for a in range(c.NKC):
                nc.vector.tensor_copy(
                    v_sb[b][a][:].rearrange("p (h e) -> p h e", e=65)[:, :, 64:65],
                    ones_f[:, 0:1].to_broadcast((128, c.NHC, 1)))

        isc = 1.0 / float(np.sqrt(c.HD))

        def attn_qblock(b, qb):
            """Both heads of a q-block together: the two 64-row score
            matmuls sit in different PE row-groups (base partitions 0/64)
            and run concurrently; each key-block's AV matmuls are emitted
            after the NEXT block's scores so the PE never queue-blocks on
            an exp. y and the softmax denominators are written
            unnormalized (normalization happens post-a2a)."""
            hp = 0
            qsl = slice(qb * c.TCH, (qb + 1) * c.TCH)
            na = (qb + 1) * c.NDIAG
            psy = [ps_y.tile([65, c.TCH], F32, tag="y", name=f"psy{h}")
                   for h in range(c.NHC)]

            def avs(a, ssbs):
                for h in range(c.NHC):
                    mm(psy[h], v_sb[b][a][:, h * 65:h * 65 + 65],
                       ssbs[h][:], a == 0, a == na - 1)

            pend = None
            for a in range(na):
                d = a - qb * c.NDIAG
                ssbs = []
                for h in range(c.NHC):
                    rs = slice(h * 64, (h + 1) * 64)
                    pss = ps_s.tile([128, c.TCH], F32, tag="s", name="pss")
                    mm(pss, kT[b][hp][rs, a * 128:(a + 1) * 128],
                       qT[b][hp][rs, qsl], True, True)
                    ssb = sp.tile([128, c.TCH], BF16, tag="ssb", name="ssb")
                    nc.scalar.activation(ssb[:], pss[:], AF.Exp, scale=isc)
                    if d >= 0:
                        # zero the above-diagonal region (DVE mask multiply)
                        nc.vector.tensor_tensor(ssb[:], ssb[:], cmask[d][:],
                                                ALU.mult)
                    ssbs.append(ssb)
                if pend is not None:
                    avs(a - 1, pend)
                pend = ssbs
            avs(na - 1, pend)
            # normalize y: both reciprocals first, then both broadcast
            # matmuls (second head's in the ps_s pool so they don't
            # serialize on the 1-bank prb ring), then copies and mults —
            # the PE queue parks only once on the reciprocal latency
            rcps = []
            for h in range(c.NHC):
                rcp = small.tile([1, c.TCH], F32, tag=f"rcp{h}",
                                 name="rcp")
                with nc.allow_low_precision(reason="feeds f32r broadcast mm"):
                    nc.vector.reciprocal(rnd(rcp), psy[h][64:65, :])
                rcps.append(rcp)
            rpss = []
            for h, pool, tag in ((0, prb, "rb"), (1, ps_s, "s")):
                rps = pool.tile([128, c.TCH], F32, tag=tag,
                                name="rps")[:64, :]
                mm(rps, ones_r[:, :64], rcps[h], True, True)
                rpss.append(rps)
            for h in range(c.NHC):
                rs = slice(h * 64, (h + 1) * 64)
                rb = bcast.tile([64, c.TCH], F32, tag=f"rb{h}", name="rb")
                nc.vector.tensor_copy(rb[:], rpss[h][:])
                nc.vector.tensor_tensor(yT[b][hp][rs, qsl], psy[h][0:64, :],
                                        rb[:], ALU.mult)

        def _emit_a2a(b):
            if getattr(c, "single", False):
                nc.sync.dma_start(a2a_out[b][:], a2a_in[b][:])
            else:
                nc.gpsimd.collective_compute(
                    "AllToAll", ALU.bypass,
                    replica_groups=[list(range(c.NC))],
                    ins=[a2a_in[b][:].opt()], outs=[a2a_out[b][:].opt()])

        xT = io["xT"]  # [D, B*T] bf16
        ncq = max(1, c.HC // 128)
        for b in range(c.B):
            for t in range(c.NTL):
                tsl = slice(b * c.T + t * c.LCH, b * c.T + (t + 1) * c.LCH)
                lsl = slice(t * c.LCH, (t + 1) * c.LCH)
                xc = xcp.tile([128, c.KD, c.LCH], BF16, tag="xc")
                nc.sync.dma_start(xc[:], rearr(xT)[:, :, tsl])
                negmu, std_, A_ = ln_stats(xc, c.LCH, xsqp, "1")

                # q / k / v projections on RAW x; LN folded as rank-1
                # terms: proj = A * (x@W + (-mu) (x) wsum + std (x) bias).
                # All projection matmuls are emitted BEFORE the Ab
                # broadcast matmul (which waits on the stats chain) so the
                # PE queue never stalls on it.
                # the 24 feature matmuls depend only on xc/weights; the six
                # rank-1 matmuls wait on the stats chain, so they trail the
                # whole group — by then negmu/std are long ready and the PE
                # queue never parks
                pss_qkv = []
                for wt in (wq, wk, wv):
                    ps = pmm.tile([128, c.TCH], F32, tag="mm",
                                  name="psqkv")[:nqp, :c.LCH]
                    for o in range(c.KD):
                        mm(ps, wt[:, o, :nqp], xc[:, o, :], o == 0, False)
                    pss_qkv.append(ps)
                for ps, (wst, bt) in zip(pss_qkv,
                                         ((wsq, bqr), (wsk, bkr),
                                          (wsv, bvr))):
                    mm(ps, wst[:, :nqp], negmu, False, False)
                    mm(ps, bt[:, :nqp], std_, False, True)

                Ab = bcast.tile([128, c.LCH], F32, tag="Ab", name="Ab")
                pbcast(Ab, A_, c.LCH, pool=ps_s, tag="s")
                for (ps, dst) in zip(pss_qkv[:2], (qT, kT)):
                    nc.vector.tensor_tensor(rnd(dst[b][0][:nqp, lsl]),
                                            ps, Ab[:nqp], ALU.mult)
                vT = vtp.tile([128, c.LCH], F32, tag="vT", name="vT")[:nqp]
                nc.vector.tensor_tensor(rnd(vT), pss_qkv[2], Ab[:nqp],
                                        ALU.mult)
                # PE-transpose v into token-major v_sb tiles
                for s2 in range(c.LCH // 128):
                    a = t * (c.LCH // 128) + s2
                    pst = pmm.tile([128, c.TCH], F32, tag="mm",
                                   name="pst")[:, :nqp]
                    nc.tensor.transpose(pst, vT[:, s2 * 128:(s2 + 1) * 128],
                                        ident[:nqp, :nqp])
                    v3 = v_sb[b][a][:].rearrange("p (h e) -> p h e",
                                                 e=65)[:, :, 0:64]
                    p3 = pst.rearrange("p (h e) -> p h e", e=64)
                    nc.vector.tensor_copy(v3, p3)

                # attention q-blocks that became ready with this chunk
                for qb in range(t * c.LCH // c.TCH,
                                ((t + 1) * c.LCH) // c.TCH):
                    attn_qblock(b, qb)
                    # this q-block's y columns are final: ship its TQH-token
                    # shards to batch b's a2a bounce buffer
                    for sj in range(c.TCH // TQH):
                        j = (qb * c.TCH) // TQH + sj
                        nc.sync.dma_start(
                            a2a_in[b][c.HC * j:c.HC * (j + 1), :],
                            yT[b][0][:, j * TQH:(j + 1) * TQH])

                # spread the phase-4 weight prefetch over chunk boundaries
                # all prefetches land before the (1,1) collective so they
                # are not queued behind its 28us Pool-queue occupancy
                ci = b * c.NTL + t
                nden = c.NTL + 1
                for pi in range(min(npre, ci * npre // nden),
                                min(npre, (ci + 1) * npre // nden)):
                    prefetch[pi]()

                if b == 1 and t == 1:
                    # batch 0's y shards are long shipped; issue its exchange
                    # now (emitting it earlier would park the collective's
                    # input wait at the Pool queue head and stall batch 1's
                    # broadcasts/selects behind it)
                    _emit_a2a(0)
                if b == 1 and t == 2:
                    # stage batch 0's phase-4 inputs while its collective
                    # result is fresh — at the phase boundary the SP queue
                    # would otherwise serialize these in front of out-proj
                    nc.sync.dma_start(yfull[0][:], rearr(a2a_out[0][:]))
                    nc.sync.dma_start(xq[:], rearr(io["xqT"]))

        # batch 1's exchange: its input wait resolves immediately (the last
        # y shards just shipped), so it doesn't head-block the Pool queue
        _emit_a2a(1)

    # ================= phase 4: out-proj, LN2, MLP =================
    # token columns 0:TQH are my batch-0 tokens, TQH:2*TQH my batch-1
    # tokens. out-proj/LN2/fc1 run batch-0 columns first so batch 1's
    # collective hides behind them; fc2 runs over all columns at once.
    with ExitStack() as ph4:
        big = ph4.enter_context(tc.tile_pool(name="big", bufs=1))
        wstr = ph4.enter_context(tc.tile_pool(name="wstr", bufs=2))
        xsqp4 = ph4.enter_context(tc.tile_pool(name="xsqp4", bufs=2))
        outp = ph4.enter_context(tc.tile_pool(name="outp", bufs=2))
        pfc2 = ph4.enter_context(tc.tile_pool(name="pfc2", bufs=1, space="PSUM"))

        # batch 1's phase-4 inputs (batch 0's were staged mid-phase-1)
        nc.sync.dma_start(yfull[1][:], rearr(a2a_out[1][:]))

        x2 = big.tile([128, c.KD, c.TQ], F32, tag="x2")
        x2n = big.tile([128, c.KD, c.TQ], BF16, tag="x2n")
        h_sb = big.tile([128, c.FD, c.TQ], BF16, tag="h")

        for bh in range(c.B):
            hsl = slice(bh * TQH, (bh + 1) * TQH)
            # out-projection + residual -> x2 (this batch's columns)
            for o in range(c.KD):
                ps = pmm.tile([128, c.TCH], F32, tag="mm",
                              name="pso")[:, :TQH]
                for k in range(c.KD):
                    mm(ps, wo_sb[:, k, o * 128:(o + 1) * 128],
                       yfull[bh][:, k, :], k == 0, k == c.KD - 1)
                nc.vector.scalar_tensor_tensor(rnd(x2[:, o, hsl]), ps,
                                               bo[:, o:o + 1],
                                               xq[:, o, hsl],
                                               ALU.add, ALU.add)

            # LN2 -> normalized bf16 activations for fc1
            ln_stats_and_apply(x2[:, :, hsl], TQH, x2n[:, :, hsl],
                               xsqp4, "2")

            # fc1 + gelu -> h
            for hidx in range(c.FD):
                ps = pmm.tile([128, c.TCH], F32, tag="mm",
                              name="psf")[:, :TQH]
                for o in range(c.KD):
                    mm(ps, wf1_sb[:, o, hidx * 128:(hidx + 1) * 128],
                       x2n[:, o, hsl], o == 0, o == c.KD - 1)
                nc.scalar.activation(h_sb[:, hidx, hsl], ps,
                                     AF.Gelu_apprx_tanh,
                                     bias=bf1[:, hidx:hidx + 1])

        # fc2 + residual -> out
        for dcb in range(c.D // c.OCB):
            nb = c.OCB // 128
            psums = [pfc2.tile([128, c.TQ], F32, tag=f"fc2_{i}", name=f"fc2_{i}")
                     for i in range(nb)]
            KHB = min(8, c.FD)
            wf2r = io["wf2"].rearrange("(o p) d -> p o d", p=128)
            for khb in range(c.FD // KHB):
                wf2_t = wstr.tile([128, KHB, c.OCB], BF16, tag="wbig")
                nc.sync.dma_start(
                    wf2_t[:],
                    wf2r[:, khb * KHB:(khb + 1) * KHB,
                         dcb * c.OCB:(dcb + 1) * c.OCB])
                for k2 in range(KHB):
                    kh = khb * KHB + k2
                    for j in range(nb):
                        mm(psums[j], wf2_t[:, k2, j * 128:(j + 1) * 128],
                           h_sb[:, kh, :], kh == 0, kh == c.FD - 1)
            for j in range(nb):
                o = dcb * nb + j
                ot = outp.tile([128, c.TQ], F32, tag="ot", name="ot")
                nc.vector.scalar_tensor_tensor(ot[:], psums[j], bf2[:, o:o + 1],
                                               x2[:, o, :], ALU.add, ALU.add)
                nc.sync.dma_start(rearr(io["out"])[:, o, :], ot[:])


# ---------------- host-side sharding ----------------

def pack_pf(v, D):
    """[D] per-feature vector -> [128, D//128] with [p, o] = v[128*o + p]."""
    return np.ascontiguousarray(np.asarray(v, np.float32).reshape(D // 128, 128).T)


def _bf16(a):
    import ml_dtypes
    return np.ascontiguousarray(np.asarray(a, np.float32).astype(ml_dtypes.bfloat16))


def make_in_maps(inputs, cfg):
    c = cfg
    x = np.asarray(inputs["x"], np.float32)
    w_qkv = np.asarray(inputs["w_qkv"], np.float32)
    b_qkv = np.asarray(inputs["b_qkv"], np.float32)
    w_o = np.ascontiguousarray(np.asarray(inputs["w_o"], np.float32))
    w_fc1 = np.ascontiguousarray(np.asarray(inputs["w_fc1"], np.float32))
    w_fc2 = np.ascontiguousarray(np.asarray(inputs["w_fc2"], np.float32))
    D = c.D

    xT_all = np.concatenate([x[b].T for b in range(c.B)], axis=1)
    xT_all = _bf16(xT_all)  # [D, B*T]

    # fold LN affine into projection weights: LN(x) = xn0*g + b with
    # xn0=(x-mu)/std; xn0 @ (g*W) + (b@W + bias) == LN(x) @ W + bias
    g1 = np.asarray(inputs["ln1_g"], np.float32)
    b1 = np.asarray(inputs["ln1_b"], np.float32)
    g2 = np.asarray(inputs["ln2_g"], np.float32)
    b2 = np.asarray(inputs["ln2_b"], np.float32)
    w_qkv_f = w_qkv * g1[:, None]
    b_qkv_f = b_qkv + b1 @ w_qkv
    w_fc1_f = np.ascontiguousarray(w_fc1 * g2[:, None])
    b_fc1_f = np.asarray(inputs["b_fc1"], np.float32) + b2 @ w_fc1

    w_o_b = _bf16(w_o)
    w_fc1_b = _bf16(w_fc1_f)
    w_fc2_b = _bf16(w_fc2)


    TQH = c.T // c.NC
    in_maps = []
    for core in range(c.NC):
        hc0 = core * c.HC                   # global head-col base of this core
        qs, ks, vs = hc0, D + hc0, 2 * D + hc0
        rows = slice(core * TQH, (core + 1) * TQH)
        m = {
            "xT": xT_all,
            # phase-4 residual: my TQH tokens of batch 0, then of batch 1
            "xqT": _bf16(np.concatenate([x[0, rows, :].T, x[1, rows, :].T],
                                        axis=1)),
            "wq": _bf16(w_qkv_f[:, qs:qs + c.HC]),
            "wk": _bf16(w_qkv_f[:, ks:ks + c.HC]),
            "wv": _bf16(w_qkv_f[:, vs:vs + c.HC]),
            "pv": _bf16(np.concatenate([
                b_qkv_f[None, qs:qs + c.HC],
                b_qkv_f[None, ks:ks + c.HC],
                b_qkv_f[None, vs:vs + c.HC],
                w_qkv_f[:, qs:qs + c.HC].sum(0, keepdims=True),
                w_qkv_f[:, ks:ks + c.HC].sum(0, keepdims=True),
                w_qkv_f[:, vs:vs + c.HC].sum(0, keepdims=True),
            ], axis=1)),
            "wo": w_o_b,
            "pm": np.concatenate([
                pack_pf(inputs["b_o"], D),
                pack_pf(b_fc1_f, c.DFF),
                pack_pf(inputs["b_fc2"], D),
            ], axis=1).astype(np.float32),
            "wf1": w_fc1_b,
            "wf2": w_fc2_b,
        }
        in_maps.append(m)
    return in_maps


def assemble_output(results, cfg):
    c = cfg
    TQH = c.T // c.NC
    out = np.empty((c.B, c.T, c.D), np.float32)
    for core in range(c.NC):
        for b in range(c.B):
            out[b, core * TQH:(core + 1) * TQH, :] = \
                results[core]["out"][:, b * TQH:(b + 1) * TQH].T
    return out


def build_nc(cfg, reps=1):
    nc = bacc.Bacc("TRN2", target_bir_lowering=False, debug=False,
                   num_devices=cfg.NC, name="nn_block")
    c = cfg
    io = {}
    specs = {
        "xT": ((c.D, c.B * c.T), BF16), "xqT": ((c.D, c.TQ), BF16),
        "wq": ((c.D, c.HC), BF16), "wk": ((c.D, c.HC), BF16),
        "wv": ((c.D, c.HC), BF16),
        "pv": ((1, 6 * c.HC), BF16),
        "pm": ((128, 2 * c.KD + c.FD), F32),
        "wo": ((c.D, c.D), BF16),
        "wf1": ((c.D, c.DFF), BF16),
        "wf2": ((c.DFF, c.D), BF16),
    }
    for name, (shape, dt) in specs.items():
        io[name] = nc.declare_dram_parameter(name, list(shape), dt,
                                             isOutput=False).ap()
    io["out"] = nc.declare_dram_parameter("out", [c.D, c.TQ], F32,
                                          isOutput=True).ap()
    with tile.TileContext(nc) as tc:
        for _ in range(reps):
            with ExitStack() as ctx:
                emit(ctx, tc, io, cfg)
    nc.compile()
    return nc


_CACHE = {}


def kernel(**inputs) -> np.ndarray:
    from concourse.bass_utils import run_bass_kernel_spmd
    cfg = CFG
    if "nc" not in _CACHE:
        _CACHE["nc"] = build_nc(cfg)
    nc = _CACHE["nc"]
    in_maps = make_in_maps(inputs, cfg)
    res = run_bass_kernel_spmd(nc, in_maps, core_ids=list(range(cfg.NC)))
    return assemble_output(res.results, cfg)
